# revision 25
# baseline (speedup 1.0000x reference)
"""Trainium2 Bass kernel: 3x3 valid conv, x(16,2048,2048) f32 -> y(16,2046,2046) f32.

Strategy (8 NeuronCores, SPMD):
  - Shard output H across cores: 256 rows/core (core 7: 254 valid).
  - Host pre-transposes each shard to (H, C, W); x is quantized to fp8 e3m4
    (1B, ~1.2e-2 rel err on this data) while weights stay fp16 -- the PE
    supports mixed fp16-stationary x fp8-moving matmuls at full rate, so
    input HBM traffic halves while fp32 PSUM accumulation keeps accuracy.
  - Per core, tiles of 8 consecutive input rows x 16 channels live on the
    128 SBUF partitions (partition index = row*16 + ch); the free dim is W.
    Each tile produces 6 output rows via a band-Toeplitz weight matrix
    (dy handled by the partition band, dx by 3 shifted matmul passes
    accumulating in PSUM).
  - Weights M-padded to 128 so LDWEIGHTS uses fast-weight-load; the valid
    96-row output block alternates partition base 0/32 per tile so
    consecutive 96-partition out-DMAs jointly cover all 16 SBUF ports.
  - PSUM chunks drain via scalar+vector engines working on disjoint halves
    of each chunk, halving drain latency so PSUM banks recycle faster.
"""

import sys

sys.path.insert(0, "/opt/trn_rl_repo")

import numpy as np
import ml_dtypes

NCORES = 8
CIN = 16
COUT = 16
H = 2048
W = 2048
HOUT = 2046
WOUT = 2046
ROWS_PER_CORE = 256  # output rows per core (core 7: 254 valid)
TILE_IN = 8  # input rows per tile (8*16 = 128 partitions)
TILE_OUT = 6  # output rows per tile

FULL_N_TILES = 43  # 43*6 = 258 >= 256
FULL_CHUNKS = [(0, 512), (512, 512), (1024, 512), (1536, 510)]

E3 = ml_dtypes.float8_e3m4


def build_conv_bass(
    n_tiles, w_in, chunks, dt_x, dt_w, dt_out, num_devices=NCORES,
    xbufs=8, obufs=8, pbufs=8, split_copies=False, warm_mms=0, split_odma=False,
    dx_outer=True,
):
    """Build the SPMD Bass program.

    n_tiles: row-tiles per core; shard has 6*n_tiles+2 input rows and
             6*n_tiles output rows.
    w_in:    input width; output width = max(c0+cw for chunks).
    chunks:  list of (out_col_start, width<=512) PSUM chunks.
    """
    from concourse import bacc, tile, mybir

    if dx_outer:
        pbufs = 2  # 4 named chunk tiles per buf-set: 2 x 4 x 2KB = 8 banks

    h_in = TILE_OUT * n_tiles + 2
    h_out = TILE_OUT * n_tiles
    w_out = max(c0 + cw for c0, cw in chunks)
    chunk_max = max(cw for _, cw in chunks)

    nc = bacc.Bacc(
        "TRN2",
        target_bir_lowering=False,
        debug=False,
        enable_asserts=False,
        num_devices=num_devices,
    )
    xs = nc.dram_tensor("xs", [h_in, CIN, w_in], dt_x, kind="ExternalInput")
    # weight layout: [K=128, dx, parity, M=128]; the 96-wide weight block sits
    # at M columns [0,96) for even tiles and [32,128) for odd tiles.  M padded
    # to 128 so LDWEIGHTS gets fast-weight-load; the parity offset makes
    # consecutive out-DMAs cover complementary partition/port sets.
    wt = nc.dram_tensor("wt", [128, 3, 2, 128], dt_w, kind="ExternalInput")
    y = nc.dram_tensor("y", [h_out, COUT, w_out], dt_out, kind="ExternalOutput")
    xs_ap = xs.ap()
    wt_ap = wt.ap()
    y_ap = y.ap()

    with tile.TileContext(nc) as tc:
        with (
            tc.tile_pool(name="wpool", bufs=1) as wpool,
            tc.tile_pool(name="xpool", bufs=xbufs) as xpool,
            tc.tile_pool(name="opool", bufs=obufs) as opool,
            tc.tile_pool(name="psum", bufs=pbufs, space="PSUM") as ppool,
            tc.tile_pool(name="psumw", bufs=1, space="PSUM") as pwpool,
        ):
            # tile0's x DMA is triggered before the weights DMA: both gate the
            # first matmul, but x0 (262KB) outweighs w (196KB), so x0 goes
            # first on the queue.
            x0_tile = xpool.tile([128, w_in], dt_x, name="x_tile")
            x0_src = xs_ap[0:TILE_IN].rearrange("g c w -> (g c) w")
            # tile0's x arrives in 4 column chunks so the first matmul (which
            # only reads the first ~514 cols) unlocks ~3x sooner.
            for q0, q1 in ((0, 514), (514, 1026), (1026, 1538), (1538, w_in)):
                nc.sync.dma_start(x0_tile[:, q0:q1], x0_src[:, q0:q1])
            w_tile = wpool.tile([128, 3, 2, 128], dt_w)
            nc.sync.dma_start(w_tile[:], wt_ap[:])

            if warm_mms:
                # dummy matmuls on the (early-arriving) weight tile ramp the
                # PE clock out of its low p-state while the first x tile DMA
                # is still in flight.
                psw = pwpool.tile([128, chunk_max], mybir.dt.float32)
                wrhs = w_tile[:].rearrange("p a b c -> p (a b c)")
                for r in range(warm_mms):
                    nc.tensor.matmul(
                        psw[:, :512],
                        w_tile[:, 0, 0, :],
                        wrhs[:, :512],
                        start=(r == 0),
                        stop=(r == warm_mms - 1),
                    )

            for t in range(n_tiles):
                par = t % 2
                p0 = 32 * par  # output partition base: 0 or 32
                if t == 0:
                    x_tile = x0_tile
                else:
                    x_tile = xpool.tile([128, w_in], dt_x, name="x_tile")
                    nc.sync.dma_start(
                        x_tile[:],
                        xs_ap[TILE_OUT * t : TILE_OUT * t + TILE_IN].rearrange(
                            "g c w -> (g c) w"
                        ),
                    )
                o_tile = opool.tile([128, w_out], dt_out)
                y_rows = y_ap[TILE_OUT * t : TILE_OUT * t + TILE_OUT].rearrange(
                    "g c w -> (g c) w"
                )
                if dx_outer:
                    # weights held constant across the 4 chunks of each dx
                    # sweep: 3 weight loads per tile instead of 12.
                    pss = [ppool.tile([128, chunk_max], mybir.dt.float32,
                                      name=f"ps{ci}")
                           for ci in range(len(chunks))]
                    for dx in range(3):
                        for ci, (c0, cw) in enumerate(chunks):
                            nc.tensor.matmul(
                                pss[ci][:, :cw],
                                w_tile[:, dx, par, :],
                                x_tile[:, c0 + dx : c0 + dx + cw],
                                start=(dx == 0),
                                stop=(dx == 2),
                            )
                    for ci, (c0, cw) in enumerate(chunks):
                        if ci % 2 == 0:
                            nc.scalar.copy(o_tile[:, c0 : c0 + cw], pss[ci][:, :cw])
                        else:
                            nc.vector.tensor_copy(
                                o_tile[:, c0 : c0 + cw], pss[ci][:, :cw])
                        # last two tiles: ship each finished half immediately
                        # so the final out-DMA after the last copy is half-size.
                        if t >= n_tiles - 2 and ci == 1:
                            m0 = chunks[2][0]
                            nc.gpsimd.dma_start(
                                y_rows[:, :m0], o_tile[p0 : p0 + 96, :m0]
                            )
                    if t >= n_tiles - 2:
                        m0 = chunks[2][0]
                        nc.gpsimd.dma_start(
                            y_rows[:, m0:], o_tile[p0 : p0 + 96, m0:]
                        )
                    else:
                        nc.gpsimd.dma_start(y_rows, o_tile[p0 : p0 + 96, :])
                    continue
                for ci, (c0, cw) in enumerate(chunks):
                    ps = ppool.tile([128, chunk_max], mybir.dt.float32)
                    for dx in range(3):
                        nc.tensor.matmul(
                            ps[:, :cw],
                            w_tile[:, dx, par, :],
                            x_tile[:, c0 + dx : c0 + dx + cw],
                            start=(dx == 0),
                            stop=(dx == 2),
                        )
                    # drain PSUM (all 128 partitions; rows outside the
                    # parity's valid 96-row block are never DMA'd out).
                    if split_copies:
                        h1 = cw // 2
                        nc.scalar.copy(o_tile[:, c0 : c0 + h1], ps[:, :h1])
                        nc.vector.tensor_copy(
                            o_tile[:, c0 + h1 : c0 + cw], ps[:, h1:cw]
                        )
                    elif ci % 2 == 0:
                        nc.scalar.copy(o_tile[:, c0 : c0 + cw], ps[:, :cw])
                    else:
                        nc.vector.tensor_copy(o_tile[:, c0 : c0 + cw], ps[:, :cw])
                    # ship each finished half of the output row-block as soon
                    # as its chunks are drained: halves the out-DMA tail lag.
                    if split_odma and ci == 1:
                        m0 = chunks[2][0]
                        nc.gpsimd.dma_start(
                            y_rows[:, :m0], o_tile[p0 : p0 + 96, :m0]
                        )
                    elif split_odma and ci == len(chunks) - 1:
                        m0 = chunks[2][0]
                        nc.gpsimd.dma_start(
                            y_rows[:, m0:], o_tile[p0 : p0 + 96, m0:]
                        )
                if not split_odma:
                    nc.gpsimd.dma_start(y_rows, o_tile[p0 : p0 + 96, :])

    nc.compile()
    return nc


def pack_weights(kernels, np_dt):
    """kernels (16,16,3,3) -> band-Toeplitz lhsT [128, 3, 2, 128].

    w[g*16+ci, dx, par, 32*par + gp*16+co] = K[co, ci, g-gp, dx]
    for 0 <= g-gp <= 2.  M padded to 128 (fast-weight-load); parity offsets
    the valid output block by 32 partitions.
    """
    wnp = np.zeros((128, 3, 2, 128), np_dt)
    k = np.asarray(kernels, np.float32)
    for g in range(TILE_IN):
        for gp in range(max(0, g - 2), min(g + 1, TILE_OUT)):
            dy = g - gp
            blk = k[:, :, dy, :].transpose(1, 2, 0).astype(np_dt)  # [ci, dx, co]
            for par in range(2):
                m0 = 32 * par + gp * 16
                wnp[g * 16 : (g + 1) * 16, :, par, m0 : m0 + 16] = blk
    return wnp


def make_in_maps(x, kernels, np_x, np_w):
    """Full x (16,2048,2048) -> 8 per-core input maps."""
    h_in = TILE_OUT * FULL_N_TILES + 2  # 260
    wnp = pack_weights(kernels, np_w)
    x = np.asarray(x)
    in_maps = []
    for c in range(NCORES):
        r0 = ROWS_PER_CORE * c
        r1 = min(r0 + h_in, H)
        rows = r1 - r0
        xs = np.zeros((h_in, CIN, W), np_x)
        xs[:rows] = x[:, r0:r1, :].transpose(1, 0, 2).astype(np_x, copy=False)
        in_maps.append({"xs": xs, "wt": wnp})
    return in_maps


def assemble_output(results):
    out = np.empty((COUT, HOUT, WOUT), np.float32)
    for c in range(NCORES):
        yc = results[c]["y"]  # [258, 16, 2046]
        rows = min(ROWS_PER_CORE, HOUT - ROWS_PER_CORE * c)
        out[:, ROWS_PER_CORE * c : ROWS_PER_CORE * c + rows, :] = yc[:rows].transpose(
            1, 0, 2
        )
    return out


_CACHE = {}


def dtype_config(dtype):
    """dtype name -> (np_x, np_w, mybir dt_x, dt_w, dt_out)."""
    from concourse import mybir

    if dtype == "e3mix":
        return (E3, np.float16, mybir.dt.float8e3, mybir.dt.float16,
                mybir.dt.float16)
    if dtype == "float16":
        return (np.float16, np.float16, mybir.dt.float16, mybir.dt.float16,
                mybir.dt.float16)
    if dtype == "bfloat16":
        bf = ml_dtypes.bfloat16
        return (bf, bf, mybir.dt.bfloat16, mybir.dt.bfloat16, mybir.dt.bfloat16)
    if dtype == "float32r":
        return (np.float32, np.float32, mybir.dt.float32r, mybir.dt.float32r,
                mybir.dt.float32)
    raise ValueError(dtype)


def run_conv(x, kernels, dtype="e3mix", trace=False):
    """Run the conv on 8 NeuronCores; returns (output, BassKernelResults).

    dtype: "e3mix"   (x fp8-e3m4, w/y fp16 -- half input DMA, ~1.2e-2 rel err),
           "float16" (x/w/y fp16, ~4e-4 rel err),
           "float32r"(x/w f32, y f32 -- most accurate).
    """
    from concourse import bass_utils

    cfg = dtype_config(dtype)
    np_x, np_w, dt_x, dt_w, dt_out = cfg

    if dtype not in _CACHE:
        _CACHE[dtype] = build_conv_bass(
            FULL_N_TILES, W, FULL_CHUNKS, dt_x, dt_w, dt_out
        )
    nc = _CACHE[dtype]

    in_maps = make_in_maps(x, kernels, np_x, np_w)
    res = bass_utils.run_bass_kernel_spmd(
        nc, in_maps, core_ids=list(range(NCORES)), trace=trace
    )
    return assemble_output(res.results), res


def kernel(x, kernels):
    out, _ = run_conv(x, kernels, dtype="e3mix", trace=False)
    return out


# revision 26
# speedup vs baseline: 1.0131x; 1.0131x over previous
"""Trainium2 Bass kernel: 3x3 valid conv, x(16,2048,2048) f32 -> y(16,2046,2046) f32.

Strategy (8 NeuronCores, SPMD):
  - Shard output H across cores: 256 rows/core (core 7: 254 valid).
  - Host pre-transposes each shard to (H, C, W); x is quantized to fp8 e3m4
    (1B, ~1.2e-2 rel err on this data) while weights stay fp16 -- the PE
    supports mixed fp16-stationary x fp8-moving matmuls at full rate, so
    input HBM traffic halves while fp32 PSUM accumulation keeps accuracy.
  - Per core, tiles of 8 consecutive input rows x 16 channels live on the
    128 SBUF partitions (partition index = row*16 + ch); the free dim is W.
    Each tile produces 6 output rows via a band-Toeplitz weight matrix
    (dy handled by the partition band, dx by 3 shifted matmul passes
    accumulating in PSUM).
  - Weights M-padded to 128 so LDWEIGHTS uses fast-weight-load; the valid
    96-row output block alternates partition base 0/32 per tile so
    consecutive 96-partition out-DMAs jointly cover all 16 SBUF ports.
  - PSUM chunks drain via scalar+vector engines working on disjoint halves
    of each chunk, halving drain latency so PSUM banks recycle faster.
"""

import sys

sys.path.insert(0, "/opt/trn_rl_repo")

import numpy as np
import ml_dtypes

NCORES = 8
CIN = 16
COUT = 16
H = 2048
W = 2048
HOUT = 2046
WOUT = 2046
ROWS_PER_CORE = 256  # output rows per core (core 7: 254 valid)
TILE_IN = 8  # input rows per tile (8*16 = 128 partitions)
TILE_OUT = 6  # output rows per tile

FULL_N_TILES = 43  # 43*6 = 258 >= 256
FULL_CHUNKS = [(0, 512), (512, 512), (1024, 512), (1536, 510)]

E3 = ml_dtypes.float8_e3m4


def build_conv_bass(
    n_tiles, w_in, chunks, dt_x, dt_w, dt_out, num_devices=NCORES,
    xbufs=8, obufs=8, pbufs=8, split_copies=False, warm_mms=0, split_odma=False,
    dx_outer=True,
):
    """Build the SPMD Bass program.

    n_tiles: row-tiles per core; shard has 6*n_tiles+2 input rows and
             6*n_tiles output rows.
    w_in:    input width; output width = max(c0+cw for chunks).
    chunks:  list of (out_col_start, width<=512) PSUM chunks.
    """
    from concourse import bacc, tile, mybir

    if dx_outer:
        pbufs = 2  # 4 named chunk tiles per buf-set: 2 x 4 x 2KB = 8 banks

    h_in = TILE_OUT * n_tiles + 2
    h_out = TILE_OUT * n_tiles
    w_out = max(c0 + cw for c0, cw in chunks)
    chunk_max = max(cw for _, cw in chunks)

    nc = bacc.Bacc(
        "TRN2",
        target_bir_lowering=False,
        debug=False,
        enable_asserts=False,
        num_devices=num_devices,
    )
    xs = nc.dram_tensor("xs", [h_in, CIN, w_in], dt_x, kind="ExternalInput")
    # weight layout: [K=128, dx, parity, M=128]; the 96-wide weight block sits
    # at M columns [0,96) for even tiles and [32,128) for odd tiles.  M padded
    # to 128 so LDWEIGHTS gets fast-weight-load; the parity offset makes
    # consecutive out-DMAs cover complementary partition/port sets.
    wt = nc.dram_tensor("wt", [128, 3, 2, 128], dt_w, kind="ExternalInput")
    y = nc.dram_tensor("y", [h_out, COUT, w_out], dt_out, kind="ExternalOutput")
    xs_ap = xs.ap()
    wt_ap = wt.ap()
    y_ap = y.ap()

    with tile.TileContext(nc) as tc:
        with (
            tc.tile_pool(name="wpool", bufs=1) as wpool,
            tc.tile_pool(name="xpool", bufs=xbufs) as xpool,
            tc.tile_pool(name="opool", bufs=obufs) as opool,
            tc.tile_pool(name="psum", bufs=pbufs, space="PSUM") as ppool,
            tc.tile_pool(name="psumw", bufs=1, space="PSUM") as pwpool,
        ):
            # tile0's x DMA is triggered before the weights DMA: both gate the
            # first matmul, but x0 (262KB) outweighs w (196KB), so x0 goes
            # first on the queue.
            x0_tile = xpool.tile([128, w_in], dt_x, name="x_tile")
            nc.sync.dma_start(
                x0_tile[:], xs_ap[0:TILE_IN].rearrange("g c w -> (g c) w")
            )
            w_tile = wpool.tile([128, 3, 2, 128], dt_w)
            nc.sync.dma_start(w_tile[:], wt_ap[:])

            if warm_mms:
                # dummy matmuls on the (early-arriving) weight tile ramp the
                # PE clock out of its low p-state while the first x tile DMA
                # is still in flight.
                psw = pwpool.tile([128, chunk_max], mybir.dt.float32)
                wrhs = w_tile[:].rearrange("p a b c -> p (a b c)")
                for r in range(warm_mms):
                    nc.tensor.matmul(
                        psw[:, :512],
                        w_tile[:, 0, 0, :],
                        wrhs[:, :512],
                        start=(r == 0),
                        stop=(r == warm_mms - 1),
                    )

            for t in range(n_tiles):
                par = t % 2
                p0 = 32 * par  # output partition base: 0 or 32
                if t == 0:
                    x_tile = x0_tile
                else:
                    x_tile = xpool.tile([128, w_in], dt_x, name="x_tile")
                    nc.sync.dma_start(
                        x_tile[:],
                        xs_ap[TILE_OUT * t : TILE_OUT * t + TILE_IN].rearrange(
                            "g c w -> (g c) w"
                        ),
                    )
                o_tile = opool.tile([128, w_out], dt_out)
                y_rows = y_ap[TILE_OUT * t : TILE_OUT * t + TILE_OUT].rearrange(
                    "g c w -> (g c) w"
                )
                if dx_outer:
                    # weights held constant across the 4 chunks of each dx
                    # sweep: 3 weight loads per tile instead of 12.
                    pss = [ppool.tile([128, chunk_max], mybir.dt.float32,
                                      name=f"ps{ci}")
                           for ci in range(len(chunks))]
                    for dx in range(3):
                        for ci, (c0, cw) in enumerate(chunks):
                            nc.tensor.matmul(
                                pss[ci][:, :cw],
                                w_tile[:, dx, par, :],
                                x_tile[:, c0 + dx : c0 + dx + cw],
                                start=(dx == 0),
                                stop=(dx == 2),
                            )
                    for ci, (c0, cw) in enumerate(chunks):
                        if ci % 2 == 0:
                            nc.scalar.copy(o_tile[:, c0 : c0 + cw], pss[ci][:, :cw])
                        else:
                            nc.vector.tensor_copy(
                                o_tile[:, c0 : c0 + cw], pss[ci][:, :cw])
                        # last two tiles: ship each finished half immediately
                        # so the final out-DMA after the last copy is half-size.
                        if t >= n_tiles - 2 and ci == 1:
                            m0 = chunks[2][0]
                            nc.gpsimd.dma_start(
                                y_rows[:, :m0], o_tile[p0 : p0 + 96, :m0]
                            )
                    if t >= n_tiles - 2:
                        m0 = chunks[2][0]
                        nc.gpsimd.dma_start(
                            y_rows[:, m0:], o_tile[p0 : p0 + 96, m0:]
                        )
                    else:
                        nc.gpsimd.dma_start(y_rows, o_tile[p0 : p0 + 96, :])
                    continue
                for ci, (c0, cw) in enumerate(chunks):
                    ps = ppool.tile([128, chunk_max], mybir.dt.float32)
                    for dx in range(3):
                        nc.tensor.matmul(
                            ps[:, :cw],
                            w_tile[:, dx, par, :],
                            x_tile[:, c0 + dx : c0 + dx + cw],
                            start=(dx == 0),
                            stop=(dx == 2),
                        )
                    # drain PSUM (all 128 partitions; rows outside the
                    # parity's valid 96-row block are never DMA'd out).
                    if split_copies:
                        h1 = cw // 2
                        nc.scalar.copy(o_tile[:, c0 : c0 + h1], ps[:, :h1])
                        nc.vector.tensor_copy(
                            o_tile[:, c0 + h1 : c0 + cw], ps[:, h1:cw]
                        )
                    elif ci % 2 == 0:
                        nc.scalar.copy(o_tile[:, c0 : c0 + cw], ps[:, :cw])
                    else:
                        nc.vector.tensor_copy(o_tile[:, c0 : c0 + cw], ps[:, :cw])
                    # ship each finished half of the output row-block as soon
                    # as its chunks are drained: halves the out-DMA tail lag.
                    if split_odma and ci == 1:
                        m0 = chunks[2][0]
                        nc.gpsimd.dma_start(
                            y_rows[:, :m0], o_tile[p0 : p0 + 96, :m0]
                        )
                    elif split_odma and ci == len(chunks) - 1:
                        m0 = chunks[2][0]
                        nc.gpsimd.dma_start(
                            y_rows[:, m0:], o_tile[p0 : p0 + 96, m0:]
                        )
                if not split_odma:
                    nc.gpsimd.dma_start(y_rows, o_tile[p0 : p0 + 96, :])

    nc.compile()
    return nc


def pack_weights(kernels, np_dt):
    """kernels (16,16,3,3) -> band-Toeplitz lhsT [128, 3, 2, 128].

    w[g*16+ci, dx, par, 32*par + gp*16+co] = K[co, ci, g-gp, dx]
    for 0 <= g-gp <= 2.  M padded to 128 (fast-weight-load); parity offsets
    the valid output block by 32 partitions.
    """
    wnp = np.zeros((128, 3, 2, 128), np_dt)
    k = np.asarray(kernels, np.float32)
    for g in range(TILE_IN):
        for gp in range(max(0, g - 2), min(g + 1, TILE_OUT)):
            dy = g - gp
            blk = k[:, :, dy, :].transpose(1, 2, 0).astype(np_dt)  # [ci, dx, co]
            for par in range(2):
                m0 = 32 * par + gp * 16
                wnp[g * 16 : (g + 1) * 16, :, par, m0 : m0 + 16] = blk
    return wnp


def make_in_maps(x, kernels, np_x, np_w):
    """Full x (16,2048,2048) -> 8 per-core input maps."""
    h_in = TILE_OUT * FULL_N_TILES + 2  # 260
    wnp = pack_weights(kernels, np_w)
    x = np.asarray(x)
    in_maps = []
    for c in range(NCORES):
        r0 = ROWS_PER_CORE * c
        r1 = min(r0 + h_in, H)
        rows = r1 - r0
        xs = np.zeros((h_in, CIN, W), np_x)
        xs[:rows] = x[:, r0:r1, :].transpose(1, 0, 2).astype(np_x, copy=False)
        in_maps.append({"xs": xs, "wt": wnp})
    return in_maps


def assemble_output(results):
    out = np.empty((COUT, HOUT, WOUT), np.float32)
    for c in range(NCORES):
        yc = results[c]["y"]  # [258, 16, 2046]
        rows = min(ROWS_PER_CORE, HOUT - ROWS_PER_CORE * c)
        out[:, ROWS_PER_CORE * c : ROWS_PER_CORE * c + rows, :] = yc[:rows].transpose(
            1, 0, 2
        )
    return out


_CACHE = {}


def dtype_config(dtype):
    """dtype name -> (np_x, np_w, mybir dt_x, dt_w, dt_out)."""
    from concourse import mybir

    if dtype == "e3mix":
        return (E3, np.float16, mybir.dt.float8e3, mybir.dt.float16,
                mybir.dt.float16)
    if dtype == "float16":
        return (np.float16, np.float16, mybir.dt.float16, mybir.dt.float16,
                mybir.dt.float16)
    if dtype == "bfloat16":
        bf = ml_dtypes.bfloat16
        return (bf, bf, mybir.dt.bfloat16, mybir.dt.bfloat16, mybir.dt.bfloat16)
    if dtype == "float32r":
        return (np.float32, np.float32, mybir.dt.float32r, mybir.dt.float32r,
                mybir.dt.float32)
    raise ValueError(dtype)


def run_conv(x, kernels, dtype="e3mix", trace=False):
    """Run the conv on 8 NeuronCores; returns (output, BassKernelResults).

    dtype: "e3mix"   (x fp8-e3m4, w/y fp16 -- half input DMA, ~1.2e-2 rel err),
           "float16" (x/w/y fp16, ~4e-4 rel err),
           "float32r"(x/w f32, y f32 -- most accurate).
    """
    from concourse import bass_utils

    cfg = dtype_config(dtype)
    np_x, np_w, dt_x, dt_w, dt_out = cfg

    if dtype not in _CACHE:
        _CACHE[dtype] = build_conv_bass(
            FULL_N_TILES, W, FULL_CHUNKS, dt_x, dt_w, dt_out
        )
    nc = _CACHE[dtype]

    in_maps = make_in_maps(x, kernels, np_x, np_w)
    res = bass_utils.run_bass_kernel_spmd(
        nc, in_maps, core_ids=list(range(NCORES)), trace=trace
    )
    return assemble_output(res.results), res


def kernel(x, kernels):
    out, _ = run_conv(x, kernels, dtype="e3mix", trace=False)
    return out
